# revision 19
# baseline (speedup 1.0000x reference)
"""Trainium2 Bass kernel for IntegratedJODIELayer (scatter_memory).

Strategy (8 NeuronCores, SPMD single program):
  - memory table [1M,128] row-sharded: core k owns rows [125000k,125000(k+1)).
    Each core copies its 64MB shard input->output on device (DRAM->DRAM DMA),
    then indirect-DMA scatters the updated rows over it.
  - events data-parallel: core k computes the MLP chains for events
    [2048k, 2048(k+1)) -> out rows (feature-major matmuls, PSUM K-chunk accum).
  - scatter routing on host: last-write-wins winner per touched memory row
    (src updates then dst updates, later event index wins), each winner chain
    assigned to the shard-owner core which recomputes it (compute is cheap and
    overlaps the shard copy; avoids cross-core collectives in one launch).
    Winner columns mix user/item branches: device computes both weight paths
    through the branch-specific layers and mask-selects before the shared
    projection chain.
  - padding winner columns carry sentinel row index INT32_MAX and are dropped
    by the indirect DMA bounds check.
"""

import sys

for _p in ("/opt/trn_rl_repo",):
    if _p not in sys.path:
        sys.path.insert(0, _p)

import numpy as np

B = 16384
NN = 1_000_000
NCORES = 8
SH = NN // NCORES          # 125000 rows per shard
EV = B // NCORES           # 2048 events per core
D = 128                    # memory/node/edge dim
TD = 100                   # time dim
TW = 512                   # event-tile width (matmul free dim)
C = 4608                   # winner-column capacity per shard (9 * 512)
SENTINEL = np.int32(2**31 - 1)
COPY_CHUNKS = 8            # shard copy granularity (15625 rows = 8MB each)

_CACHE = {}


def _legalize_sync_waits(bir: dict) -> dict:
    """This walrus build encodes at most ONE sync wait per instruction
    (setupSyncWait: 'Too many sync wait commands'). Hoist all but the last
    wait of every instruction into standalone single-wait EventSemaphore
    instructions on the same engine, immediately preceding it."""
    for fn in bir.get("functions", []):
        for blk in fn.get("blocks", []):
            insts = blk.get("instructions")
            if not insts:
                continue
            out = []
            for ins in insts:
                si = ins.get("sync_info")
                waits = (si or {}).get("on_wait") or []
                if len(waits) > 1:
                    for j, w in enumerate(waits[:-1]):
                        out.append({
                            "debug": ins.get("debug", 0),
                            "engine": ins["engine"],
                            "ins": [],
                            "name": f"{ins['name']}__w{j}",
                            "opcode": "EventSemaphore",
                            "outs": [],
                            "sync_info": {"on_update": [], "on_wait": [w]},
                        })
                    si["on_wait"] = [waits[-1]]
                out.append(ins)
            blk["instructions"] = out
    return bir


def _patch_serialization(nc):
    import json

    orig = nc.to_json_bytes

    def patched():
        bir = json.loads(orig())
        return json.dumps(_legalize_sync_waits(bir)).encode()

    nc.to_json_bytes = patched
    return nc


def _build_nc():
    from concourse import bass, mybir
    import concourse.tile as tile
    from concourse.masks import make_identity

    f32 = mybir.dt.float32
    i32 = mybir.dt.int32
    AF = mybir.ActivationFunctionType

    nc = bass.Bass()

    # ---- DRAM I/O ----
    mem = nc.dram_tensor("mem", [SH, D], f32, kind="ExternalInput")
    sembT = nc.dram_tensor("sembT", [D, EV], f32, kind="ExternalInput")
    dembT = nc.dram_tensor("dembT", [D, EV], f32, kind="ExternalInput")
    edgeT = nc.dram_tensor("edgeT", [D, EV], f32, kind="ExternalInput")
    srcmT = nc.dram_tensor("srcmT", [D, EV], f32, kind="ExternalInput")
    dstmT = nc.dram_tensor("dstmT", [D, EV], f32, kind="ExternalInput")
    tsE = nc.dram_tensor("tsE", [TD, EV], f32, kind="ExternalInput")

    wA = nc.dram_tensor("wA", [D, C], f32, kind="ExternalInput")
    wB = nc.dram_tensor("wB", [D, C], f32, kind="ExternalInput")
    wEdge = nc.dram_tensor("wEdge", [D, C], f32, kind="ExternalInput")
    wTs = nc.dram_tensor("wTs", [TD, C], f32, kind="ExternalInput")
    wMask = nc.dram_tensor("wMask", [D, C], f32, kind="ExternalInput")
    wIdx = nc.dram_tensor("wIdx", [C, 1], i32, kind="ExternalInput")

    Wu1 = nc.dram_tensor("Wu1", [484, D], f32, kind="ExternalInput")
    Wu2 = nc.dram_tensor("Wu2", [D, D], f32, kind="ExternalInput")
    Wi1 = nc.dram_tensor("Wi1", [484, D], f32, kind="ExternalInput")
    Wi2 = nc.dram_tensor("Wi2", [D, D], f32, kind="ExternalInput")
    Wp1 = nc.dram_tensor("Wp1", [D, D], f32, kind="ExternalInput")
    Wp2 = nc.dram_tensor("Wp2", [D, D], f32, kind="ExternalInput")
    Wt1 = nc.dram_tensor("Wt1", [D + TD, D], f32, kind="ExternalInput")
    Wo = nc.dram_tensor("Wo", [2 * D, D], f32, kind="ExternalInput")
    # cols 0-7: bu1,bu2,bi1,bi2,bp1,bp2,bt1,bo; col 8: tw; col 9: tb + pi/2
    BIAS = nc.dram_tensor("BIAS", [D, 10], f32, kind="ExternalInput")

    new_mem = nc.dram_tensor("new_mem", [SH, D], f32, kind="ExternalOutput")
    oSrcT = nc.dram_tensor("oSrcT", [D, EV], f32, kind="ExternalOutput")
    oDstT = nc.dram_tensor("oDstT", [D, EV], f32, kind="ExternalOutput")

    with tile.TileContext(nc) as tc:
        with (
            tc.tile_pool(name="const", bufs=1) as cp,
            tc.tile_pool(name="work", bufs=4) as wp,
            tc.tile_pool(name="scat", bufs=8) as sp,
            tc.tile_pool(name="ps_mm", bufs=6, space="PSUM") as ps_mm,
            tc.tile_pool(name="ps_tr", bufs=2, space="PSUM") as ps_tr,
        ):
            # ---- shard pass-through copy (the memory-roofline term) ----
            rows_per = SH // COPY_CHUNKS
            for ci in range(COPY_CHUNKS):
                r0 = ci * rows_per
                r1 = SH if ci == COPY_CHUNKS - 1 else r0 + rows_per
                nc.sync.dma_start(out=new_mem[r0:r1, :], in_=mem[r0:r1, :])

            # ---- constants in SBUF ----
            ident = cp.tile([128, 128], f32, tag="ident")
            make_identity(nc, ident[:])

            def load_w_chunks(dram, koffs, tag):
                tiles = []
                for j, (k0, kn) in enumerate(koffs):
                    t = cp.tile([128, D], f32, tag=f"{tag}{j}")
                    nc.sync.dma_start(out=t[:kn, :], in_=dram[k0:k0 + kn, :])
                    tiles.append((t, kn))
                return tiles

            cat4 = [(0, 128), (128, 128), (256, 128), (384, TD)]
            wu1 = load_w_chunks(Wu1, cat4, "wu1")
            wi1 = load_w_chunks(Wi1, cat4, "wi1")
            wu2 = load_w_chunks(Wu2, [(0, 128)], "wu2")
            wi2 = load_w_chunks(Wi2, [(0, 128)], "wi2")
            wp1 = load_w_chunks(Wp1, [(0, 128)], "wp1")
            wp2 = load_w_chunks(Wp2, [(0, 128)], "wp2")
            wt1 = load_w_chunks(Wt1, [(0, 128), (128, TD)], "wt1")
            wo = load_w_chunks(Wo, [(0, 128), (128, 128)], "wo")

            bias = cp.tile([D, 10], f32, tag="bias")
            nc.sync.dma_start(out=bias[:], in_=BIAS[:])
            BU1, BU2, BI1, BI2, BP1, BP2, BT1, BO = (bias[:, j:j + 1] for j in range(8))
            tw_s = bias[:TD, 8:9]
            tb_s = bias[:TD, 9:10]

            def mm(psum_ap, pairs):
                n = len(pairs)
                for j, ((w, kn), rhs) in enumerate(pairs):
                    nc.tensor.matmul(out=psum_ap, lhsT=w[:kn, :], rhs=rhs[:kn, :],
                                     start=(j == 0), stop=(j == n - 1))

            def layer(pairs, func, b, tag, width, pdim=D):
                """matmul-accumulate pairs into PSUM, then ACT func(x+bias)->SBUF."""
                p = ps_mm.tile([pdim, width], f32, tag="pmm")
                mm(p[:, :], pairs)
                o = wp.tile([pdim, width], f32, tag=tag)
                nc.scalar.activation(o[:, :], p[:, :], func, bias=b)
                return o

            def time_feats(ts_dram, col0, width):
                # ts pre-broadcast to TD partitions host-side;
                # cos(t*w + b) = Sin(t*scale + bias), pi/2 folded into tb_s
                ts_s = wp.tile([TD, width], f32, tag="tss")
                nc.sync.dma_start(out=ts_s[:, :], in_=ts_dram[:, col0:col0 + width])
                o = wp.tile([TD, width], f32, tag="timeT")
                nc.scalar.activation(o[:, :], ts_s[:, :], AF.Sin,
                                     bias=tb_s, scale=tw_s)
                return o

            def load(dram, col0, width, tag, pdim=D):
                t = wp.tile([pdim, width], f32, tag=tag)
                nc.sync.dma_start(out=t[:, :], in_=dram[:, col0:col0 + width])
                return t

            # ---- part 1: out projections for this core's event slice ----
            for t in range(EV // TW):
                c0 = t * TW
                tf = time_feats(tsE, c0, TW)
                sm = load(srcmT, c0, TW, "sm")
                dm = load(dstmT, c0, TW, "dm")
                eg = load(edgeT, c0, TW, "eg")
                se = load(sembT, c0, TW, "se")
                de = load(dembT, c0, TW, "de")

                hu = layer([(wu1[0], sm), (wu1[1], dm), (wu1[2], eg), (wu1[3], tf)],
                           AF.Relu, BU1, "hu", TW)
                uu = layer([(wu2[0], hu)], AF.Identity, BU2, "uu", TW)
                hi = layer([(wi1[0], dm), (wi1[1], sm), (wi1[2], eg), (wi1[3], tf)],
                           AF.Relu, BI1, "hi", TW)
                ui = layer([(wi2[0], hi)], AF.Identity, BI2, "ui", TW)

                def tbatch(u, tfeat):
                    h = layer([(wp1[0], u)], AF.Relu, BP1, "hp", TW)
                    pr = layer([(wp2[0], h)], AF.Identity, BP2, "pr", TW)
                    return layer([(wt1[0], pr), (wt1[1], tfeat)], AF.Relu, BT1, "tb", TW)

                ut = tbatch(uu, tf)
                it = tbatch(ui, tf)
                osrc = layer([(wo[0], ut), (wo[1], se)], AF.Identity, BO, "osrc", TW)
                odst = layer([(wo[0], it), (wo[1], de)], AF.Identity, BO, "odst", TW)
                nc.sync.dma_start(out=oSrcT[:, c0:c0 + TW], in_=osrc[:, :])
                nc.sync.dma_start(out=oDstT[:, c0:c0 + TW], in_=odst[:, :])

            # ---- part 2: winner chains + scatter into own shard ----
            for t in range(C // TW):
                c0 = t * TW
                tf = time_feats(wTs, c0, TW)
                a = load(wA, c0, TW, "wa")
                b2 = load(wB, c0, TW, "wb")
                eg = load(wEdge, c0, TW, "weg")
                mk = load(wMask, c0, TW, "wmk")

                hu = layer([(wu1[0], a), (wu1[1], b2), (wu1[2], eg), (wu1[3], tf)],
                           AF.Relu, BU1, "hu", TW)
                uu = layer([(wu2[0], hu)], AF.Identity, BU2, "uu", TW)
                hi = layer([(wi1[0], a), (wi1[1], b2), (wi1[2], eg), (wi1[3], tf)],
                           AF.Relu, BI1, "hi", TW)
                ui = layer([(wi2[0], hi)], AF.Identity, BI2, "ui", TW)

                # select branch-specific update: sel = ui + mask*(uu - ui)
                dlt = wp.tile([D, TW], f32, tag="dlt")
                nc.vector.tensor_sub(dlt[:, :], uu[:, :], ui[:, :])
                nc.vector.tensor_mul(dlt[:, :], dlt[:, :], mk[:, :])
                sel = wp.tile([D, TW], f32, tag="sel")
                nc.vector.tensor_add(sel[:, :], dlt[:, :], ui[:, :])

                h = layer([(wp1[0], sel)], AF.Relu, BP1, "hp", TW)
                pr = layer([(wp2[0], h)], AF.Identity, BP2, "pr", TW)
                tb2 = layer([(wt1[0], pr), (wt1[1], tf)], AF.Relu, BT1, "tb", TW)

                for j in range(TW // 128):
                    ptr = ps_tr.tile([128, 128], f32, tag="ptr")
                    nc.tensor.transpose(out=ptr[:, :], in_=tb2[:, j * 128:(j + 1) * 128],
                                        identity=ident[:])
                    rows = sp.tile([128, D], f32, tag="rows")
                    nc.vector.tensor_copy(rows[:, :], ptr[:, :])
                    ix = sp.tile([128, 1], i32, tag="ix")
                    nc.sync.dma_start(out=ix[:, :],
                                      in_=wIdx[c0 + j * 128:c0 + (j + 1) * 128, :])
                    nc.gpsimd.indirect_dma_start(
                        out=new_mem[:],
                        out_offset=bass.IndirectOffsetOnAxis(ap=ix[:, :1], axis=0),
                        in_=rows[:, :],
                        in_offset=None,
                        bounds_check=SH - 1,
                        oob_is_err=False,
                    )
    return _patch_serialization(nc)


def _route(sid, did):
    """Last-write-wins winner per touched row.

    Stream = all src updates (branch 0) then all dst updates (branch 1), in
    event order; the later stream position wins, matching
    memory.at[src].set(...).at[dst].set(...) applied update-by-update.
    Returns (rows, branch, event) arrays of unique winner rows.
    """
    allr = np.concatenate([sid, did])
    allbr = np.concatenate([np.zeros(B, np.int8), np.ones(B, np.int8)])
    alle = np.concatenate([np.arange(B), np.arange(B)])
    rev = allr[::-1]
    uniq, ridx = np.unique(rev, return_index=True)
    pos = len(allr) - 1 - ridx
    return uniq, allbr[pos], alle[pos]


def _host_chain(inp, e_list, br_list, out_rows, new_memory):
    """Numpy fallback: compute winner chains exactly like the reference."""
    f = np.float32
    ts = inp["timestamps"].astype(f)[e_list]
    te = np.cos(ts[:, None] * inp["tw"][None, :].astype(f) + inp["tb"].astype(f))
    sm = inp["memory"][inp["src_node_ids"][e_list]].astype(f)
    dm = inp["memory"][inp["dst_node_ids"][e_list]].astype(f)
    eg = inp["edge_features"][e_list].astype(f)
    u = br_list == 0
    A = np.where(u[:, None], sm, dm)
    Bm = np.where(u[:, None], dm, sm)
    x = np.concatenate([A, Bm, eg, te], 1)

    def mlp2(x, W1, b1, W2, b2):
        return np.maximum(x @ W1 + b1, 0) @ W2 + b2

    upd_u = mlp2(x, inp["Wu1"], inp["bu1"], inp["Wu2"], inp["bu2"])
    upd_i = mlp2(x, inp["Wi1"], inp["bi1"], inp["Wi2"], inp["bi2"])
    upd = np.where(u[:, None], upd_u, upd_i)
    pr = mlp2(upd, inp["Wp1"], inp["bp1"], inp["Wp2"], inp["bp2"])
    tb = np.maximum(np.concatenate([pr, te], 1) @ inp["Wt1"] + inp["bt1"], 0)
    new_memory[out_rows] = tb.astype(f)


def kernel(**inputs):
    from concourse.bass_utils import run_bass_kernel_spmd

    f = np.float32
    inp = {k: np.asarray(v) for k, v in inputs.items()}
    sid = inp["src_node_ids"].astype(np.int64)
    did = inp["dst_node_ids"].astype(np.int64)
    memory = np.ascontiguousarray(inp["memory"], f)
    se, de = inp["src_node_embeddings"].astype(f), inp["dst_node_embeddings"].astype(f)
    eg, ts = inp["edge_features"].astype(f), inp["timestamps"].astype(f)

    if "nc" not in _CACHE:
        _CACHE["nc"] = _build_nc()
    nc = _CACHE["nc"]

    rows, br, ev = _route(sid, did)

    biases = np.zeros((D, 10), f)
    for j, k in enumerate(("bu1", "bu2", "bi1", "bi2", "bp1", "bp2", "bt1", "bo")):
        biases[:, j] = inp[k].astype(f)
    biases[:TD, 8] = inp["tw"].astype(f)
    biases[:TD, 9] = inp["tb"].astype(f) + np.float32(np.pi / 2)
    shared = {
        "Wu1": np.ascontiguousarray(inp["Wu1"], f),
        "Wu2": np.ascontiguousarray(inp["Wu2"], f),
        "Wi1": np.ascontiguousarray(inp["Wi1"], f),
        "Wi2": np.ascontiguousarray(inp["Wi2"], f),
        "Wp1": np.ascontiguousarray(inp["Wp1"], f),
        "Wp2": np.ascontiguousarray(inp["Wp2"], f),
        "Wt1": np.ascontiguousarray(inp["Wt1"], f),
        "Wo": np.ascontiguousarray(inp["Wo"], f),
        "BIAS": np.ascontiguousarray(biases),
    }

    in_maps = []
    overflow = []  # (rows, br, ev) handled on host if a shard exceeds C
    for k in range(NCORES):
        sl = slice(k * EV, (k + 1) * EV)
        m = (rows // SH) == k
        wr, wb, we = rows[m], br[m], ev[m]
        if len(wr) > C:
            overflow.append((wr[C:], wb[C:], we[C:]))
            wr, wb, we = wr[:C], wb[:C], we[:C]
        n = len(wr)
        a_ids = np.where(wb == 0, sid[we], did[we])
        b_ids = np.where(wb == 0, did[we], sid[we])

        def padT(x, width=C):  # x [n, D] -> [D, width] f32, zero-padded
            o = np.zeros((D, width), f)
            o[:, :x.shape[0]] = x.T
            return o

        widx = np.full((C, 1), SENTINEL, np.int32)
        widx[:n, 0] = (wr - k * SH).astype(np.int32)
        wts = np.zeros((TD, C), f)
        wts[:, :n] = ts[we][None, :]
        wmask = np.zeros((D, C), f)
        wmask[:, :n] = (wb == 0).astype(f)[None, :]

        in_maps.append({
            "mem": memory[k * SH:(k + 1) * SH],
            "sembT": np.ascontiguousarray(se[sl].T),
            "dembT": np.ascontiguousarray(de[sl].T),
            "edgeT": np.ascontiguousarray(eg[sl].T),
            "srcmT": np.ascontiguousarray(memory[sid[sl]].T),
            "dstmT": np.ascontiguousarray(memory[did[sl]].T),
            "tsE": np.ascontiguousarray(np.broadcast_to(ts[sl][None, :], (TD, EV))),
            "wA": padT(memory[a_ids]),
            "wB": padT(memory[b_ids]),
            "wEdge": padT(eg[we]),
            "wTs": wts,
            "wMask": wmask,
            "wIdx": widx,
            **shared,
        })

    res = run_bass_kernel_spmd(nc, in_maps, list(range(NCORES)))
    _CACHE["last_results"] = res

    out = np.empty((2 * B, D), f)
    new_memory = np.empty((NN, D), f)
    for k in range(NCORES):
        r = res.results[k]
        new_memory[k * SH:(k + 1) * SH] = r["new_mem"]
        out[k * EV:(k + 1) * EV] = r["oSrcT"].T
        out[B + k * EV:B + (k + 1) * EV] = r["oDstT"].T

    for wr, wb, we in overflow:
        _host_chain(inp, we, wb, wr, new_memory)

    return out, new_memory
